# revision 10
# baseline (speedup 1.0000x reference)
"""Trainium2 Bass kernel for nn_AspectModel (span-attention aspect tagger).

Strategy: batch-shard the 32 sentences 4-per-core across 8 NeuronCores; route
each fragment (host-side) to the core owning its sentence, padded to 48 slots
per sentence (192 slots/core).  All heavy math runs on-chip:
  - span features (l_word / word_state / r_word) via a masks-matmul against
    the sentence hidden states (host-built one-hot + in-span masks),
  - v = span @ att_w and c = span @ att_b as dense matmuls over all slots,
  - attention scores via a PE matmul of V against the transposed memory,
  - masked softmax (fused exp+sum) and mix via a second masks-matmul,
  - tag logits + log_softmax.
Matmul operands (x, xT, att_w, tag_w, att_b, masks) are pre-cast to bf16 on
the host; f32 PSUM accumulation keeps precision.  DMA layout is tuned for the
SDMA per-packet overhead: inputs are packed into few DMAs with fat (4-11KB)
partition rows, ordered so each consumer starts as early as possible
(masks+x first, att weights in thirds for progressive vmm, transposed memory
last).  The memory transpose is host-precomputed (the on-chip xbar transpose
emits 256B packets and runs 3x slower than streaming it from HBM).
Each core returns its own [2, 96, 5] slot outputs; the host scatters them
back into the full [1024, 5] output.  No collectives needed.
"""

import sys
import types

import ml_dtypes
import numpy as np

# Optional shim so run_bass_kernel_spmd(trace=True) works in containers where
# antenv.axon_hooks is missing (profiling only; correctness path unaffected).
try:
    import antenv.axon_hooks  # noqa: F401
except ImportError:
    try:
        from trn_agent_boot.trn_boot import _ntff_profile_via_ctypes

        _hook = _ntff_profile_via_ctypes("/opt/axon/libaxon_pjrt.so")
        _mod = types.ModuleType("antenv.axon_hooks")
        _mod.get_axon_ntff_profile_hook = lambda: _hook
        _mod.set_axon_ntff_profile_hook = lambda h: None
        sys.modules["antenv.axon_hooks"] = _mod
    except Exception:
        pass

import concourse.bass as bass  # noqa: E402
import concourse.tile as tile  # noqa: E402
from concourse import bacc, mybir  # noqa: E402
from concourse import bass_utils  # noqa: E402
from concourse.bass_utils import run_bass_kernel_spmd  # noqa: E402

# No artifact bucket in the sandbox; make tracing's upload step a no-op.
bass_utils.upload_artifacts = lambda tmpdir: f"local:{tmpdir}"

F32 = mybir.dt.float32
BF16 = mybir.dt.bfloat16
I32 = mybir.dt.int32
ALU = mybir.AluOpType
ACT = mybir.ActivationFunctionType

B, S, D, F, T = 32, 256, 512, 1024, 5
NCORES = 8
SEN = 4          # sentences per core
G = 48           # fragment slots per sentence
C = SEN * G      # 192 fragment slots per core
Q = C // 2       # 96 slots per q-half (sentence pair)
D3 = 3 * D
MKW = 2 * 4 * C      # mask columns in the sx tensor (1536)
XW = 2 * SEN * D     # x columns in the sx tensor (4096)

TRACE = False
LAST_RESULT = None  # BassKernelResults of the most recent run (for test.py)

_compiled = {}


def _build(seq_len: float):
    """Build + compile the per-core SPMD graph (identical on all 8 cores)."""
    nc = bacc.Bacc("TRN2", target_bir_lowering=False, debug=False,
                   num_devices=NCORES)

    # sx = [span masks (k, comp, slot) | x (chunk, d)] -- one fat-row DMA.
    sx_d = nc.dram_tensor("sx", [128, MKW + XW], BF16, kind="ExternalInput")
    aw_d = nc.dram_tensor("aw", [128, 12, D], BF16, kind="ExternalInput")
    # awt = [ab (12 cols) | tag_w.T packed (16*5 cols)]
    awt_d = nc.dram_tensor("awt", [128, 12 + 16 * T], BF16,
                           kind="ExternalInput")
    # meta1 cols: 0-1 fs_c, 2-3 fm_c; row 0 cols 6-10: tag_b
    meta1_d = nc.dram_tensor("meta1", [128, 12], F32, kind="ExternalInput")
    # host-transposed memory [d%128, (l, k), (dj, s')]
    xt_d = nc.dram_tensor("xt", [128, 2 * SEN, 512], BF16,
                          kind="ExternalInput")
    out_d = nc.dram_tensor("out", [128, 2, T], F32, kind="ExternalOutput")

    with tile.TileContext(nc) as tc:
        with (
            tc.tile_pool(name="persist", bufs=1) as pp,
            tc.tile_pool(name="work", bufs=2) as wp,
            tc.tile_pool(name="psum", bufs=2, space="PSUM") as psp,
        ):
            # ---- persistent SBUF tensors ----
            sx_sb = pp.tile([128, MKW + XW], BF16, tag="sx_sb")
            mkT_sb = sx_sb[:, 0:MKW].rearrange(
                "p (k c s) -> p k c s", k=2, c=4)
            mkT = [mkT_sb[:, k, 0:3, :] for k in range(2)]
            x_bf = sx_sb[:, MKW:MKW + XW].rearrange(
                "p (m d) -> p m d", m=2 * SEN)
            aw_bf = pp.tile([128, 12, D], BF16, tag="aw_bf")
            awt_bf = pp.tile([128, 12 + 16 * T], BF16, tag="awt_bf")
            meta1 = pp.tile([128, 12], F32, tag="meta1")
            iota_i = pp.tile([128, S], I32, tag="iota_i")
            iota_f = pp.tile([128, S], F32, tag="iota_f")
            iota_n = pp.tile([128, S], F32, tag="iota_n")
            spanT = pp.tile([128, 12, C], BF16, tag="spanT")
            v_sb = pp.tile([128, 4, C], BF16, tag="v_sb")
            memT = pp.tile([128, SEN, 2, 4, 128], BF16, tag="memT")
            mixT = pp.tile([128, 4, C], BF16, tag="mixT")

            # ---- input DMAs.  sync ring carries the big stream in
            # consumer-priority order; scalar ring carries the small bits.
            sx_dma = nc.sync.dma_start(sx_sb[:], sx_d.ap())
            prev = sx_dma
            for third in range(3):
                ad = nc.sync.dma_start(
                    aw_bf[:, third * 4:(third + 1) * 4, :],
                    aw_d.ap()[:, third * 4:(third + 1) * 4, :])
                tile.add_dep_helper(ad.ins, prev.ins, sync=False,
                                    reason="sync ring order")
                prev = ad
            for hf in range(2):
                xd = nc.sync.dma_start(
                    memT[:, 2 * hf:2 * hf + 2, :, :, :],
                    xt_d.ap()[:, 4 * hf:4 * hf + 4, :])
                tile.add_dep_helper(xd.ins, prev.ins, sync=False,
                                    reason="sync ring order")
                prev = xd
            m1_dma = nc.scalar.dma_start(meta1[:], meta1_d.ap())
            at_dma = nc.scalar.dma_start(awt_bf[:], awt_d.ap())
            tile.add_dep_helper(at_dma.ins, m1_dma.ins, sync=False,
                                reason="scalar ring order")

            fs_c = meta1[:, 0:2]
            fm_c = meta1[:, 2:4]
            tb_sb = meta1[0:1, 6:6 + T]

            # ---- constants ----
            neg4 = pp.tile([128, 1], F32, tag="neg4")
            nc.gpsimd.memset(neg4[:], -1.0e4)
            one1 = pp.tile([1, 1], F32, tag="one1")
            nc.gpsimd.memset(one1[:], 1.0)
            ones_bf = pp.tile([128, 1], BF16, tag="ones_bf")
            nc.gpsimd.memset(ones_bf[:], 1.0)
            ones_row = pp.tile([1, Q], BF16, tag="ones_row")
            nc.gpsimd.memset(ones_row[:], 1.0)
            tb_bf = pp.tile([1, T], BF16, tag="tb_bf")
            nc.vector.tensor_copy(tb_bf[:], tb_sb)
            # Warm the scalar-engine activation table while DMAs stream, so
            # no ACT_TABLE_LOAD lands on the critical path later.  The scalar
            # engine runs ONLY tanh/exp/ln (copies live on vector/gpsimd).
            dmy = wp.tile([1, 1], F32, tag="dmy")
            nc.scalar.activation(dmy[:], one1[:], ACT.Tanh)
            nc.scalar.activation(dmy[:], one1[:], ACT.Exp)
            nc.scalar.activation(dmy[:], one1[:], ACT.Ln)
            nc.gpsimd.iota(iota_i[:], pattern=[[1, S]], channel_multiplier=0)
            nc.vector.tensor_copy(iota_f[:], iota_i[:])
            nc.vector.tensor_scalar_mul(iota_n[:], iota_f[:], -1.0)

            # ---- span masks-matmul: spanT[3D, C] (l_word | word_state | r_word)
            # j0-major so the first half of spanT's kk chunks completes early
            # and vmm can begin while the second half is still accumulating.
            sc_span = nc.named_scope("spanmm"); sc_span.__enter__()
            for j0 in range(2):
                for l in range(SEN):
                    ps = psp.tile([128, 2, 3, G], F32, tag="psm")
                    for dj in range(2):
                        j = j0 * 2 + dj
                        for k in range(2):
                            nc.tensor.matmul(
                                ps[:, dj, :, :],
                                x_bf[:, l * 2 + k, j * 128:(j + 1) * 128],
                                mkT[k][:, :, l * G:(l + 1) * G],
                                start=(k == 0), stop=(k == 1),
                            )
                    nc.vector.tensor_copy(
                        spanT[:, j0 * 6:j0 * 6 + 6, l * G:(l + 1) * G], ps[:])

            sc_span.__exit__(None, None, None)
            # ---- c = span @ att_b as a row vector, then partition-broadcast
            # (before vmm: cmm only needs awt + spanT, fills the PE while the
            # att_w stream is still in flight)
            sc_c = nc.named_scope("cmm"); sc_c.__enter__()
            pc = psp.tile([1, C], F32, tag="pout", bufs=4)
            for kk in range(12):
                nc.tensor.matmul(
                    pc[:],
                    awt_bf[:, kk:kk + 1],
                    spanT[:, kk, :],
                    start=(kk == 0), stop=(kk == 11),
                )
            c_row = pp.tile([1, C], F32, tag="c_row")
            nc.vector.tensor_copy(c_row[:], pc[:])
            c_bc = pp.tile([128, C], F32, tag="c_bc")
            nc.gpsimd.partition_broadcast(c_bc[:], c_row[:])

            sc_c.__exit__(None, None, None)
            # ---- v = span @ att_w  (stored transposed: V[d, slot])
            sc_v = nc.named_scope("vmm"); sc_v.__enter__()
            for m0 in range(2):
                pv = psp.tile([128, 2, C], F32, tag="p2k")
                for mj in range(2):
                    m = m0 * 2 + mj
                    for kk in range(12):
                        nc.tensor.matmul(
                            pv[:, mj, :],
                            aw_bf[:, kk, m * 128:(m + 1) * 128],
                            spanT[:, kk, :],
                            start=(kk == 0), stop=(kk == 11),
                        )
                nc.vector.tensor_copy(v_sb[:, m0 * 2:m0 * 2 + 2, :], pv[:])

            sc_v.__exit__(None, None, None)
            # ---- pos-weight (pw) per pair, then transpose to [s, slot] ----
            pwT = pp.tile([128, 2, 2, Q], BF16, tag="pwT")  # [s', q, k, slot]
            for q in range(2):
                fs_q = fs_c[:, q:q + 1]
                fm_q = fm_c[:, q:q + 1]
                t1 = wp.tile([128, S], F32, tag="t1")
                t2 = wp.tile([128, S], F32, tag="t2")
                dm = wp.tile([128, S], F32, tag="dm")
                pwr = wp.tile([128, S], F32, tag="pwr")
                noti = wp.tile([128, S], F32, tag="noti")
                pwb = wp.tile([128, S], BF16, tag="pwb")
                nc.vector.tensor_scalar(t1[:], iota_n[:], fs_q, None,
                                        op0=ALU.add)            # fs - s
                nc.vector.tensor_scalar(t2[:], iota_f[:], fm_q, None,
                                        op0=ALU.subtract)       # s - (fe-1)
                nc.vector.tensor_tensor(dm[:], t1[:], t2[:], op=ALU.max)
                nc.vector.tensor_scalar(pwr[:], dm[:], -1.0 / seq_len, 1.0,
                                        op0=ALU.mult, op1=ALU.add)
                nc.vector.tensor_single_scalar(noti[:], dm[:], 0.0,
                                               op=ALU.is_gt)    # not in span
                nc.vector.tensor_tensor(pwb[:], pwr[:], noti[:], op=ALU.mult)
                # [slot, s] -> [s', k, slot]
                nc.scalar.dma_start_transpose(pwT[:, q, :, :], pwb[0:Q, :])

            # ---- per sentence-pair attention + mix + logits ----
            # scores kept transposed ([s, slot]); softmax denominator via a
            # PE ones-matmul; normalization folded into the logits scale.
            sh_t = pp.tile([128, 2, T], F32, tag="sh_t")
            se_t = pp.tile([128, 2], F32, tag="se_t")
            res_t = pp.tile([128, 2, T], F32, tag="res_t")

            # PE-early: both pairs' score matmuls and span-logits (with
            # tag_b folded in via a ones-row matmul), so the PE FIFO never
            # stalls behind the pair-0 activation chain.
            gts = []
            for q in range(2):
                gt = psp.tile([128, 2, Q], F32, tag="p2k", name=f"gt{q}")
                gts.append(gt)
                for h in range(2):
                    l = 2 * q + h
                    for k in range(2):
                        for dj in range(4):
                            nc.tensor.matmul(
                                gt[:, k, h * G:(h + 1) * G],
                                memT[:, l, k, dj, :],
                                v_sb[:, dj, l * G:(l + 1) * G],
                                start=(dj == 0), stop=(dj == 3),
                            )
            plss = []
            for q in range(2):
                pls = psp.tile([128, T], F32, tag="pout", bufs=4, name=f"pls{q}")
                for kk in range(12):
                    nc.tensor.matmul(
                        pls[0:Q, :], spanT[:, kk, q * Q:(q + 1) * Q],
                        awt_bf[:, 12 + kk * T:12 + (kk + 1) * T],
                        start=(kk == 0), stop=False)
                nc.tensor.matmul(pls[0:Q, :], ones_row[:], tb_bf[:],
                                 start=False, stop=True)
                pls_sb = pp.tile([128, T], F32, tag=f"pls_sb{q}")
                nc.vector.tensor_copy(pls_sb[0:Q, :], pls[0:Q, :])
                plss.append(pls_sb)

            for q in range(2):
                gt = gts[q]
                sg = wp.tile([128, 2, Q], F32, tag="sg")
                th = wp.tile([128, 2, Q], F32, tag="th")
                thm = wp.tile([128, 2, Q], F32, tag="thm")
                uT = wp.tile([128, 2, Q], BF16, tag="uT")
                wTu = wp.tile([128, 2, Q], BF16, tag="wTu")
                rden = wp.tile([128, 1], F32, tag="rden")

                # scores = tanh(pw * G + c); masked exp (still un-normalized)
                nc.vector.tensor_tensor(sg[:], gt[:], pwT[:, q, :, :],
                                        op=ALU.mult)
                cb = c_bc[:, q * Q:(q + 1) * Q]
                nc.vector.tensor_tensor(
                    sg[:], sg[:],
                    cb.rearrange("p (o c) -> p o c",
                                 o=1).broadcast_to([128, 2, Q]),
                    op=ALU.add)
                nc.scalar.activation(th[:], sg[:], ACT.Tanh)
                kpq = mkT_sb[:, :, 3, q * Q:(q + 1) * Q]
                nc.vector.scalar_tensor_tensor(thm[:], th[:], 1.0e4, kpq,
                                               op0=ALU.add, op1=ALU.mult)
                nc.scalar.activation(uT[:], thm[:], ACT.Exp, bias=neg4[:])

                # denominator via ones-matmul ([slot, 1] per pair)
                dn = psp.tile([128, 1], F32, tag="pout", bufs=4)
                for k in range(2):
                    nc.tensor.matmul(dn[0:Q, :], uT[:, k, :], ones_bf[:],
                                     start=(k == 0), stop=(k == 1))
                nc.vector.reciprocal(rden[0:Q, :], dn[0:Q, :])
                nc.vector.tensor_tensor(wTu[:], uT[:], pwT[:, q, :, :],
                                        op=ALU.mult)

                # mixT_unnorm[d, slot] = sum_s mem[s, d] * u[slot, s] * pw
                for h in range(2):
                    l = 2 * q + h
                    pm = psp.tile([128, 4, G], F32, tag="psm")
                    for dj in range(4):
                        for k in range(2):
                            nc.tensor.matmul(
                                pm[:, dj, :],
                                x_bf[:, l * 2 + k, dj * 128:(dj + 1) * 128],
                                wTu[:, k, h * G:(h + 1) * G],
                                start=(k == 0), stop=(k == 1),
                            )
                    nc.vector.tensor_copy(mixT[:, :, l * G:(l + 1) * G], pm[:])

                plm = psp.tile([128, T], F32, tag="pout", bufs=4)
                for dj in range(4):
                    nc.tensor.matmul(
                        plm[0:Q, :], mixT[:, dj, q * Q:(q + 1) * Q],
                        awt_bf[:, 12 + (12 + dj) * T:12 + (13 + dj) * T],
                        start=(dj == 0), stop=(dj == 3))

                # logits = pls(+tb) + rden*plm; log-softmax without the max
                # shift (logits are small; exp is safe in f32).  Whole
                # epilogue is per-q so pair 0's result DMA overlaps pair 1.
                nc.vector.scalar_tensor_tensor(
                    sh_t[0:Q, q, :], plm[0:Q, :], rden[0:Q, :],
                    plss[q][0:Q, :], op0=ALU.mult, op1=ALU.add)
                ex2 = wp.tile([128, T], F32, tag="ex2")
                nc.scalar.activation(ex2[0:Q, :], sh_t[0:Q, q, :], ACT.Exp,
                                     accum_out=se_t[0:Q, q:q + 1])
                lse1 = wp.tile([128, 1], F32, tag="lse1")
                nc.scalar.activation(lse1[0:Q, :], se_t[0:Q, q:q + 1], ACT.Ln)
                nc.vector.tensor_scalar(res_t[0:Q, q, :], sh_t[0:Q, q, :],
                                        lse1[0:Q, :], None, op0=ALU.subtract)
                nc.sync.dma_start(out_d.ap()[0:Q, q, :], res_t[0:Q, q, :])

    nc.compile()
    return nc


def _host_prep(en_output, lengths, frag_b, frag_s, frag_e, att_w, att_b,
               tag_w, tag_b):
    """Shard + relayout inputs.  Returns (in_maps, assign, overflow)."""
    # replicated weights, permuted so spanT chunk kk = 3*j + comp maps to
    # att rows comp*512 + j*128 : .. + 128.
    perm = np.concatenate([
        np.arange(comp * D + j * 128, comp * D + (j + 1) * 128)
        for j in range(4) for comp in range(3)
    ])
    aw_np = att_w[perm].reshape(12, 128, D).transpose(1, 0, 2).reshape(128, 12, D)
    ab_np = att_b[perm].reshape(12, 128).T.copy()
    tw_rows = np.concatenate([tag_w[:, perm].T,
                              tag_w[:, D3:].T], axis=0)  # [2048, 5]
    tw_np = tw_rows.reshape(16, 128, T).transpose(1, 0, 2).reshape(128, 16, T)
    tb_np = tag_b.reshape(1, T).astype(np.float32)

    aw_np = np.ascontiguousarray(aw_np).astype(ml_dtypes.bfloat16)
    awt_np = np.concatenate(
        [ab_np, tw_np.reshape(128, 16 * T)], axis=1)
    awt_np = np.ascontiguousarray(awt_np).astype(ml_dtypes.bfloat16)

    assign = np.full((F, 2), -1, dtype=np.int64)  # (core, slot) per fragment
    counts = np.zeros((NCORES, SEN), dtype=np.int64)
    overflow = []
    in_maps = []

    fs_slot = np.zeros((NCORES, C), np.float32)
    fm_slot = np.zeros((NCORES, C), np.float32)
    ln_slot = np.full((NCORES, C), float(S), np.float32)

    for i in range(F):
        b = int(frag_b[i])
        core, l = b // SEN, b % SEN
        k = counts[core, l]
        if k >= G:
            overflow.append(i)
            continue
        counts[core, l] += 1
        slot = l * G + k
        assign[i] = (core, slot)
        fs_slot[core, slot] = frag_s[i]
        fm_slot[core, slot] = frag_e[i] - 1
        ln_slot[core, slot] = lengths[b]

    for core in range(NCORES):
        xs = np.asarray(en_output[core * SEN:(core + 1) * SEN])  # [4, 256, 512]
        x_np = xs.reshape(SEN, 2, 128, D).transpose(2, 0, 1, 3) \
                 .reshape(128, 2 * SEN * D)
        # [d%128, (l, k), (dj, s')]
        xt_np = np.ascontiguousarray(
            xs.reshape(SEN, 2, 128, 4, 128).transpose(4, 0, 1, 3, 2)
              .reshape(128, 2 * SEN, 512)).astype(ml_dtypes.bfloat16)
        meta1 = np.zeros((128, 12), np.float32)
        meta1[0:Q, 0:2] = fs_slot[core].reshape(2, Q).T
        meta1[0:Q, 2:4] = fm_slot[core].reshape(2, Q).T
        meta1[0, 6:6 + T] = tb_np[0]
        # span masks [S, 4, C] -> [128, (k=2, comp=4, C)] (s = k*128 + p);
        # component 3 is the attention keep-mask (!in_span & s < len)
        pos = np.arange(S, dtype=np.float32)[:, None]
        fs = fs_slot[core][None, :]
        fm = fm_slot[core][None, :]
        ln = ln_slot[core][None, :]
        mk = np.empty((S, 4, C), np.float32)
        in_span = (pos >= fs) & (pos <= fm)
        mk[:, 0, :] = pos == fs
        mk[:, 1, :] = in_span
        mk[:, 2, :] = pos == fm
        mk[:, 3, :] = (~in_span) & (pos < ln)
        mk = mk.reshape(2, 128, 4 * C).transpose(1, 0, 2).reshape(128, 2 * 4 * C)
        sx_np = np.ascontiguousarray(np.concatenate(
            [mk, x_np], axis=1)).astype(ml_dtypes.bfloat16)
        in_maps.append({
            "sx": sx_np, "aw": aw_np, "awt": awt_np,
            "meta1": meta1, "xt": xt_np,
        })
    return in_maps, assign, overflow


def _host_fragment(en_output, lengths, s, e, b, att_w, att_b, tag_w, tag_b,
                   seq_len):
    """Numpy fallback for (vanishingly rare) slot-overflow fragments."""
    mem = en_output[b].astype(np.float64)
    ws = mem[s:e].sum(0)
    span = np.concatenate([mem[s], ws, mem[e - 1]])
    pos = np.arange(S)
    in_span = (pos >= s) & (pos < e)
    att_mask = in_span | (pos >= lengths[b])
    dis = np.where(pos < s, s - pos,
                   np.where(pos >= e, pos - e + 1, seq_len)).astype(np.float64)
    pwv = 1.0 - dis / seq_len
    fin = pwv[:, None] * mem
    v = span @ att_w.astype(np.float64)
    c = span @ att_b.astype(np.float64)
    sc = np.tanh(fin @ v + c)
    sc = np.where(att_mask, -1e4, sc)
    sc = sc - sc.max()
    a = np.exp(sc)
    a = a / a.sum()
    mix = a @ fin
    ms = np.concatenate([span, mix])
    lg = ms @ tag_w.astype(np.float64).T + tag_b.astype(np.float64)
    lg = lg - lg.max()
    return (lg - np.log(np.exp(lg).sum())).astype(np.float32)


def kernel(en_output, lengths, frag_b, frag_s, frag_e, att_w, att_b, tag_w,
           tag_b):
    global LAST_RESULT
    en_output = np.asarray(en_output, dtype=np.float32)
    lengths = np.asarray(lengths).astype(np.int64)
    frag_b = np.asarray(frag_b).astype(np.int64)
    frag_s = np.asarray(frag_s).astype(np.int64)
    frag_e = np.asarray(frag_e).astype(np.int64)
    att_w = np.asarray(att_w, dtype=np.float32)
    att_b = np.asarray(att_b, dtype=np.float32)
    tag_w = np.asarray(tag_w, dtype=np.float32)
    tag_b = np.asarray(tag_b, dtype=np.float32)

    seq_len = float(lengths[0])
    if seq_len not in _compiled:
        _compiled[seq_len] = _build(seq_len)
    nc = _compiled[seq_len]

    in_maps, assign, overflow = _host_prep(
        en_output, lengths, frag_b, frag_s, frag_e, att_w, att_b, tag_w, tag_b)

    res = run_bass_kernel_spmd(nc, in_maps, core_ids=list(range(NCORES)),
                               trace=TRACE)
    LAST_RESULT = res

    out = np.empty((F, T), dtype=np.float32)
    per_core = [res.results[i]["out"][0:Q].transpose(1, 0, 2).reshape(C, T)
                for i in range(NCORES)]
    cores = assign[:, 0]
    slots = assign[:, 1]
    for core in range(NCORES):
        sel = cores == core
        out[sel] = per_core[core][slots[sel]]
    for i in overflow:
        out[i] = _host_fragment(en_output, lengths, int(frag_s[i]),
                                int(frag_e[i]), int(frag_b[i]), att_w, att_b,
                                tag_w, tag_b, seq_len)
    return out


# revision 11
# speedup vs baseline: 1.0412x; 1.0412x over previous
"""Trainium2 Bass kernel for nn_AspectModel (span-attention aspect tagger).

Strategy: batch-shard the 32 sentences 4-per-core across 8 NeuronCores; route
each fragment (host-side) to the core owning its sentence, padded to 48 slots
per sentence (192 slots/core).  All heavy math runs on-chip:
  - span features (l_word / word_state / r_word) via a masks-matmul against
    the sentence hidden states (host-built one-hot + in-span masks),
  - v = span @ att_w and c = span @ att_b as dense matmuls over all slots,
  - attention scores via a PE matmul of V against the transposed memory,
  - masked softmax (fused exp+sum) and mix via a second masks-matmul,
  - tag logits + log_softmax.
Matmul operands (x, xT, att_w, tag_w, att_b, masks) are pre-cast to bf16 on
the host; f32 PSUM accumulation keeps precision.  DMA layout is tuned for the
SDMA per-packet overhead and the tile framework's 8 DMA-semaphore lanes:
exactly 5 input DMAs with fat (4-11KB) partition rows, ordered so each
consumer starts as early as possible (masks+x first, att weights in halves
for progressive vmm, transposed memory last).  The memory transpose is
host-precomputed (the on-chip xbar transpose emits 256B packets and runs 3x
slower than streaming from HBM).  Position weights are computed directly in
[s, slot] layout: fragment starts/ends are recovered on-chip from the
one-hot masks via tiny iota matmuls, so no meta tensor and no pw transpose
is needed.  Each core returns its own [2, 96, 5] slot outputs; the host
scatters them back into the full [1024, 5] output.  No collectives needed.
"""

import sys
import types

import ml_dtypes
import numpy as np

# Optional shim so run_bass_kernel_spmd(trace=True) works in containers where
# antenv.axon_hooks is missing (profiling only; correctness path unaffected).
try:
    import antenv.axon_hooks  # noqa: F401
except ImportError:
    try:
        from trn_agent_boot.trn_boot import _ntff_profile_via_ctypes

        _hook = _ntff_profile_via_ctypes("/opt/axon/libaxon_pjrt.so")
        _mod = types.ModuleType("antenv.axon_hooks")
        _mod.get_axon_ntff_profile_hook = lambda: _hook
        _mod.set_axon_ntff_profile_hook = lambda h: None
        sys.modules["antenv.axon_hooks"] = _mod
    except Exception:
        pass

import concourse.bass as bass  # noqa: E402
import concourse.tile as tile  # noqa: E402
from concourse import bacc, mybir  # noqa: E402
from concourse import bass_utils  # noqa: E402
from concourse.bass_utils import run_bass_kernel_spmd  # noqa: E402

# No artifact bucket in the sandbox; make tracing's upload step a no-op.
bass_utils.upload_artifacts = lambda tmpdir: f"local:{tmpdir}"

F32 = mybir.dt.float32
BF16 = mybir.dt.bfloat16
I32 = mybir.dt.int32
ALU = mybir.AluOpType
ACT = mybir.ActivationFunctionType

B, S, D, F, T = 32, 256, 512, 1024, 5
NCORES = 8
SEN = 4          # sentences per core
G = 48           # fragment slots per sentence
C = SEN * G      # 192 fragment slots per core
Q = C // 2       # 96 slots per q-half (sentence pair)
D3 = 3 * D
MKW = 2 * 4 * C      # mask columns in the sx tensor (1536)
XW = 2 * SEN * D     # x columns in the sx tensor (4096)
AWTW = 12 + 16 * T + T  # awt cols: ab | tag_w.T | tag_b row

TRACE = False
LAST_RESULT = None  # BassKernelResults of the most recent run (for test.py)

_compiled = {}


def _build(seq_len: float):
    """Build + compile the per-core SPMD graph (identical on all 8 cores)."""
    nc = bacc.Bacc("TRN2", target_bir_lowering=False, debug=False,
                   num_devices=NCORES)

    # sx = [span masks (k, comp, slot) | x (chunk, d)] -- one fat-row DMA.
    sx_d = nc.dram_tensor("sx", [128, MKW + XW], BF16, kind="ExternalInput")
    aw_d = nc.dram_tensor("aw", [128, 12, D], BF16, kind="ExternalInput")
    # awt = [ab (12) | tag_w.T packed (16*5) | row0: tag_b (5)]
    awt_d = nc.dram_tensor("awt", [128, AWTW], BF16, kind="ExternalInput")
    # host-transposed memory [d%128, (l, k), (dj, s')]
    xt_d = nc.dram_tensor("xt", [128, 2 * SEN, 512], BF16,
                          kind="ExternalInput")
    out_d = nc.dram_tensor("out", [128, 2, T], F32, kind="ExternalOutput")

    with tile.TileContext(nc) as tc:
        with (
            tc.tile_pool(name="persist", bufs=1) as pp,
            tc.tile_pool(name="work", bufs=2) as wp,
            tc.tile_pool(name="psum", bufs=2, space="PSUM") as psp,
        ):
            # ---- persistent SBUF tensors ----
            sx_sb = pp.tile([128, MKW + XW], BF16, tag="sx_sb")
            mkT_sb = sx_sb[:, 0:MKW].rearrange(
                "p (k c s) -> p k c s", k=2, c=4)
            mkT = [mkT_sb[:, k, 0:3, :] for k in range(2)]
            x_bf = sx_sb[:, MKW:MKW + XW].rearrange(
                "p (m d) -> p m d", m=2 * SEN)
            aw_bf = pp.tile([128, 12, D], BF16, tag="aw_bf")
            awt_bf = pp.tile([128, AWTW], BF16, tag="awt_bf")
            tb_bf = awt_bf[0:1, 92:92 + T]
            spanT = pp.tile([128, 12, C], BF16, tag="spanT")
            v_sb = pp.tile([128, 4, C], BF16, tag="v_sb")
            memT = pp.tile([128, SEN, 2, 4, 128], BF16, tag="memT")
            mixT = pp.tile([128, 4, C], BF16, tag="mixT")

            # ---- input DMAs.  sync ring carries the big stream in
            # consumer-priority order; scalar ring carries the small awt.
            # 5 input DMAs + 2 output DMAs keeps us inside the 8 DMA
            # semaphore lanes (a 9th DMA would stall on lane retirement).
            sx_dma = nc.sync.dma_start(sx_sb[:], sx_d.ap())
            prev = sx_dma
            for hh in range(2):
                ad = nc.sync.dma_start(
                    aw_bf[:, hh * 6:(hh + 1) * 6, :],
                    aw_d.ap()[:, hh * 6:(hh + 1) * 6, :])
                tile.add_dep_helper(ad.ins, prev.ins, sync=False,
                                    reason="sync ring order")
                prev = ad
            xd = nc.sync.dma_start(memT[:], xt_d.ap())
            tile.add_dep_helper(xd.ins, prev.ins, sync=False,
                                reason="sync ring order")
            nc.scalar.dma_start(awt_bf[:], awt_d.ap())

            # ---- constants ----
            neg4 = pp.tile([128, 1], F32, tag="neg4")
            nc.gpsimd.memset(neg4[:], -1.0e4)
            one1 = pp.tile([1, 1], F32, tag="one1")
            nc.gpsimd.memset(one1[:], 1.0)
            ones_bf = pp.tile([128, 1], BF16, tag="ones_bf")
            nc.gpsimd.memset(ones_bf[:], 1.0)
            ones_row = pp.tile([1, Q], BF16, tag="ones_row")
            nc.gpsimd.memset(ones_row[:], 1.0)
            # Warm the scalar-engine activation table while DMAs stream, so
            # no ACT_TABLE_LOAD lands on the critical path later.  The scalar
            # engine runs ONLY tanh/exp/ln (copies live on vector).
            dmy = wp.tile([1, 1], F32, tag="dmy")
            nc.scalar.activation(dmy[:], one1[:], ACT.Tanh)
            nc.scalar.activation(dmy[:], one1[:], ACT.Exp)
            nc.scalar.activation(dmy[:], one1[:], ACT.Ln)
            # iota over partitions: s_col[k] = p + 128k (f32 + exact bf16)
            iota_i = pp.tile([128, 1], I32, tag="iota_i")
            nc.gpsimd.iota(iota_i[:], pattern=[[0, 1]], channel_multiplier=1)
            s_col = pp.tile([128, 2], F32, tag="s_col")
            nc.vector.tensor_copy(s_col[:, 0:1], iota_i[:])
            nc.vector.tensor_scalar(s_col[:, 1:2], s_col[:, 0:1], 128.0, None,
                                    op0=ALU.add)
            s_bf = pp.tile([128, 2], BF16, tag="s_bf")
            nc.vector.tensor_copy(s_bf[:], s_col[:])

            # ---- fs/fm rows from the one-hot masks via iota matmuls ----
            # fs[slot] = sum_s s * (pos==fs)[s, slot]; same for fm (comp 2).
            pfs = psp.tile([1, C], F32, tag="pout", bufs=4)
            for k in range(2):
                nc.tensor.matmul(pfs[:], s_bf[:, k:k + 1],
                                 mkT_sb[:, k, 0, :],
                                 start=(k == 0), stop=(k == 1))
            pfm = psp.tile([1, C], F32, tag="pout", bufs=4)
            for k in range(2):
                nc.tensor.matmul(pfm[:], s_bf[:, k:k + 1],
                                 mkT_sb[:, k, 2, :],
                                 start=(k == 0), stop=(k == 1))
            fs_sb = pp.tile([1, C], F32, tag="fs_sb")
            nc.vector.tensor_copy(fs_sb[:], pfs[:])
            fm_sb = pp.tile([1, C], F32, tag="fm_sb")
            nc.vector.tensor_copy(fm_sb[:], pfm[:])
            fs_bc = pp.tile([128, C], F32, tag="fs_bc")
            nc.gpsimd.partition_broadcast(fs_bc[:], fs_sb[:])
            fm_bc = pp.tile([128, C], F32, tag="fm_bc")
            nc.gpsimd.partition_broadcast(fm_bc[:], fm_sb[:])

            # ---- span masks-matmul: spanT[3D, C] (l_word | word_state | r_word)
            # j0-major so the first half of spanT's kk chunks completes early
            # and vmm can begin while the second half is still accumulating.
            sc_span = nc.named_scope("spanmm"); sc_span.__enter__()
            for j0 in range(2):
                for l in range(SEN):
                    ps = psp.tile([128, 2, 3, G], F32, tag="psm")
                    for dj in range(2):
                        j = j0 * 2 + dj
                        for k in range(2):
                            nc.tensor.matmul(
                                ps[:, dj, :, :],
                                x_bf[:, l * 2 + k, j * 128:(j + 1) * 128],
                                mkT[k][:, :, l * G:(l + 1) * G],
                                start=(k == 0), stop=(k == 1),
                            )
                    nc.vector.tensor_copy(
                        spanT[:, j0 * 6:j0 * 6 + 6, l * G:(l + 1) * G], ps[:])

            sc_span.__exit__(None, None, None)
            # ---- pos-weight directly in [s', k, slot] layout (no transpose)
            pw_t = pp.tile([128, 2, C], BF16, tag="pw_t")
            for k in range(2):
                sk = s_col[:, k:k + 1]
                t1 = wp.tile([128, C], F32, tag="t1")
                t2 = wp.tile([128, C], F32, tag="t2")
                dm = wp.tile([128, C], F32, tag="dm")
                pwr = wp.tile([128, C], F32, tag="pwr")
                noti = wp.tile([128, C], F32, tag="noti")
                nc.vector.tensor_scalar(t1[:], fs_bc[:], sk, None,
                                        op0=ALU.subtract)        # fs - s
                nc.vector.tensor_scalar(t2[:], fm_bc[:], sk, -1.0,
                                        op0=ALU.subtract,
                                        op1=ALU.mult)            # s - fm
                nc.vector.tensor_tensor(dm[:], t1[:], t2[:], op=ALU.max)
                nc.vector.tensor_scalar(pwr[:], dm[:], -1.0 / seq_len, 1.0,
                                        op0=ALU.mult, op1=ALU.add)
                nc.vector.tensor_single_scalar(noti[:], dm[:], 0.0,
                                               op=ALU.is_gt)     # not in span
                nc.vector.tensor_tensor(pw_t[:, k, :], pwr[:], noti[:],
                                        op=ALU.mult)

            # ---- c = span @ att_b as a row vector, then partition-broadcast
            # (before vmm: cmm only needs awt + spanT, fills the PE while the
            # att_w stream is still in flight)
            sc_c = nc.named_scope("cmm"); sc_c.__enter__()
            pc = psp.tile([1, C], F32, tag="pout", bufs=4)
            for kk in range(12):
                nc.tensor.matmul(
                    pc[:],
                    awt_bf[:, kk:kk + 1],
                    spanT[:, kk, :],
                    start=(kk == 0), stop=(kk == 11),
                )
            c_row = pp.tile([1, C], F32, tag="c_row")
            nc.vector.tensor_copy(c_row[:], pc[:])
            c_bc = pp.tile([128, C], F32, tag="c_bc")
            nc.gpsimd.partition_broadcast(c_bc[:], c_row[:])

            sc_c.__exit__(None, None, None)
            # ---- v = span @ att_w  (stored transposed: V[d, slot])
            sc_v = nc.named_scope("vmm"); sc_v.__enter__()
            for m0 in range(2):
                pv = psp.tile([128, 2, C], F32, tag="p2k")
                for mj in range(2):
                    m = m0 * 2 + mj
                    for kk in range(12):
                        nc.tensor.matmul(
                            pv[:, mj, :],
                            aw_bf[:, kk, m * 128:(m + 1) * 128],
                            spanT[:, kk, :],
                            start=(kk == 0), stop=(kk == 11),
                        )
                nc.vector.tensor_copy(v_sb[:, m0 * 2:m0 * 2 + 2, :], pv[:])

            sc_v.__exit__(None, None, None)
            # ---- per sentence-pair attention + mix + logits ----
            # scores kept transposed ([s, slot]); softmax denominator via a
            # PE ones-matmul; normalization folded into the logits scale.
            sh_t = pp.tile([128, 2, T], F32, tag="sh_t")
            se_t = pp.tile([128, 2], F32, tag="se_t")
            res_t = pp.tile([128, 2, T], F32, tag="res_t")

            # PE-early: both pairs' score matmuls and span-logits (with
            # tag_b folded in via a ones-row matmul), so the PE FIFO never
            # stalls behind the pair-0 activation chain.
            gts = []
            for q in range(2):
                gt = psp.tile([128, 2, Q], F32, tag="p2k", name=f"gt{q}")
                gts.append(gt)
                for h in range(2):
                    l = 2 * q + h
                    for k in range(2):
                        for dj in range(4):
                            nc.tensor.matmul(
                                gt[:, k, h * G:(h + 1) * G],
                                memT[:, l, k, dj, :],
                                v_sb[:, dj, l * G:(l + 1) * G],
                                start=(dj == 0), stop=(dj == 3),
                            )
            plss = []
            for q in range(2):
                pls = psp.tile([128, T], F32, tag="pout", bufs=4, name=f"pls{q}")
                for kk in range(12):
                    nc.tensor.matmul(
                        pls[0:Q, :], spanT[:, kk, q * Q:(q + 1) * Q],
                        awt_bf[:, 12 + kk * T:12 + (kk + 1) * T],
                        start=(kk == 0), stop=False)
                nc.tensor.matmul(pls[0:Q, :], ones_row[:], tb_bf,
                                 start=False, stop=True)
                pls_sb = pp.tile([128, T], F32, tag=f"pls_sb{q}")
                nc.vector.tensor_copy(pls_sb[0:Q, :], pls[0:Q, :])
                plss.append(pls_sb)

            for q in range(2):
                gt = gts[q]
                sg = wp.tile([128, 2, Q], F32, tag="sg")
                th = wp.tile([128, 2, Q], F32, tag="th")
                thm = wp.tile([128, 2, Q], F32, tag="thm")
                uT = wp.tile([128, 2, Q], BF16, tag="uT")
                wTu = wp.tile([128, 2, Q], BF16, tag="wTu")
                rden = wp.tile([128, 1], F32, tag="rden")
                pwq = pw_t[:, :, q * Q:(q + 1) * Q]

                # scores = tanh(pw * G + c); masked exp (still un-normalized)
                nc.vector.tensor_tensor(sg[:], gt[:], pwq, op=ALU.mult)
                cb = c_bc[:, q * Q:(q + 1) * Q]
                nc.vector.tensor_tensor(
                    sg[:], sg[:],
                    cb.rearrange("p (o c) -> p o c",
                                 o=1).broadcast_to([128, 2, Q]),
                    op=ALU.add)
                nc.scalar.activation(th[:], sg[:], ACT.Tanh)
                kpq = mkT_sb[:, :, 3, q * Q:(q + 1) * Q]
                nc.vector.scalar_tensor_tensor(thm[:], th[:], 1.0e4, kpq,
                                               op0=ALU.add, op1=ALU.mult)
                nc.scalar.activation(uT[:], thm[:], ACT.Exp, bias=neg4[:])

                # denominator via ones-matmul ([slot, 1] per pair)
                dn = psp.tile([128, 1], F32, tag="pout", bufs=4)
                for k in range(2):
                    nc.tensor.matmul(dn[0:Q, :], uT[:, k, :], ones_bf[:],
                                     start=(k == 0), stop=(k == 1))
                nc.vector.reciprocal(rden[0:Q, :], dn[0:Q, :])
                nc.vector.tensor_tensor(wTu[:], uT[:], pwq, op=ALU.mult)

                # mixT_unnorm[d, slot] = sum_s mem[s, d] * u[slot, s] * pw
                for h in range(2):
                    l = 2 * q + h
                    pm = psp.tile([128, 4, G], F32, tag="psm")
                    for dj in range(4):
                        for k in range(2):
                            nc.tensor.matmul(
                                pm[:, dj, :],
                                x_bf[:, l * 2 + k, dj * 128:(dj + 1) * 128],
                                wTu[:, k, h * G:(h + 1) * G],
                                start=(k == 0), stop=(k == 1),
                            )
                    nc.vector.tensor_copy(mixT[:, :, l * G:(l + 1) * G], pm[:])

                plm = psp.tile([128, T], F32, tag="pout", bufs=4)
                for dj in range(4):
                    nc.tensor.matmul(
                        plm[0:Q, :], mixT[:, dj, q * Q:(q + 1) * Q],
                        awt_bf[:, 12 + (12 + dj) * T:12 + (13 + dj) * T],
                        start=(dj == 0), stop=(dj == 3))

                # logits = pls(+tb) + rden*plm; log-softmax without the max
                # shift (logits are small; exp is safe in f32).  Whole
                # epilogue is per-q so pair 0's result DMA overlaps pair 1.
                nc.vector.scalar_tensor_tensor(
                    sh_t[0:Q, q, :], plm[0:Q, :], rden[0:Q, :],
                    plss[q][0:Q, :], op0=ALU.mult, op1=ALU.add)
                ex2 = wp.tile([128, T], F32, tag="ex2")
                nc.scalar.activation(ex2[0:Q, :], sh_t[0:Q, q, :], ACT.Exp,
                                     accum_out=se_t[0:Q, q:q + 1])
                lse1 = wp.tile([128, 1], F32, tag="lse1")
                nc.scalar.activation(lse1[0:Q, :], se_t[0:Q, q:q + 1], ACT.Ln)
                nc.vector.tensor_scalar(res_t[0:Q, q, :], sh_t[0:Q, q, :],
                                        lse1[0:Q, :], None, op0=ALU.subtract)
                nc.sync.dma_start(out_d.ap()[0:Q, q, :], res_t[0:Q, q, :])

    nc.compile()
    return nc


def _host_prep(en_output, lengths, frag_b, frag_s, frag_e, att_w, att_b,
               tag_w, tag_b):
    """Shard + relayout inputs.  Returns (in_maps, assign, overflow)."""
    # replicated weights, permuted so spanT chunk kk = 3*j + comp maps to
    # att rows comp*512 + j*128 : .. + 128.
    perm = np.concatenate([
        np.arange(comp * D + j * 128, comp * D + (j + 1) * 128)
        for j in range(4) for comp in range(3)
    ])
    aw_np = att_w[perm].reshape(12, 128, D).transpose(1, 0, 2).reshape(128, 12, D)
    ab_np = att_b[perm].reshape(12, 128).T.copy()
    tw_rows = np.concatenate([tag_w[:, perm].T,
                              tag_w[:, D3:].T], axis=0)  # [2048, 5]
    tw_np = tw_rows.reshape(16, 128, T).transpose(1, 0, 2).reshape(128, 16, T)

    aw_np = np.ascontiguousarray(aw_np).astype(ml_dtypes.bfloat16)
    awt_np = np.zeros((128, AWTW), np.float32)
    awt_np[:, 0:12] = ab_np
    awt_np[:, 12:92] = tw_np.reshape(128, 16 * T)
    awt_np[0, 92:92 + T] = tag_b
    awt_np = np.ascontiguousarray(awt_np).astype(ml_dtypes.bfloat16)

    assign = np.full((F, 2), -1, dtype=np.int64)  # (core, slot) per fragment
    counts = np.zeros((NCORES, SEN), dtype=np.int64)
    overflow = []
    in_maps = []

    fs_slot = np.zeros((NCORES, C), np.float32)
    fm_slot = np.zeros((NCORES, C), np.float32)
    ln_slot = np.full((NCORES, C), float(S), np.float32)

    for i in range(F):
        b = int(frag_b[i])
        core, l = b // SEN, b % SEN
        k = counts[core, l]
        if k >= G:
            overflow.append(i)
            continue
        counts[core, l] += 1
        slot = l * G + k
        assign[i] = (core, slot)
        fs_slot[core, slot] = frag_s[i]
        fm_slot[core, slot] = frag_e[i] - 1
        ln_slot[core, slot] = lengths[b]

    for core in range(NCORES):
        xs = np.asarray(en_output[core * SEN:(core + 1) * SEN])  # [4, 256, 512]
        x_np = xs.reshape(SEN, 2, 128, D).transpose(2, 0, 1, 3) \
                 .reshape(128, 2 * SEN * D)
        # [d%128, (l, k), (dj, s')]
        xt_np = np.ascontiguousarray(
            xs.reshape(SEN, 2, 128, 4, 128).transpose(4, 0, 1, 3, 2)
              .reshape(128, 2 * SEN, 512)).astype(ml_dtypes.bfloat16)
        # span masks [S, 4, C] -> [128, (k=2, comp=4, C)] (s = k*128 + p);
        # component 3 is the attention keep-mask (!in_span & s < len)
        pos = np.arange(S, dtype=np.float32)[:, None]
        fs = fs_slot[core][None, :]
        fm = fm_slot[core][None, :]
        ln = ln_slot[core][None, :]
        mk = np.empty((S, 4, C), np.float32)
        in_span = (pos >= fs) & (pos <= fm)
        mk[:, 0, :] = pos == fs
        mk[:, 1, :] = in_span
        mk[:, 2, :] = pos == fm
        mk[:, 3, :] = (~in_span) & (pos < ln)
        mk = mk.reshape(2, 128, 4 * C).transpose(1, 0, 2).reshape(128, 2 * 4 * C)
        sx_np = np.ascontiguousarray(np.concatenate(
            [mk, x_np], axis=1)).astype(ml_dtypes.bfloat16)
        in_maps.append({
            "sx": sx_np, "aw": aw_np, "awt": awt_np, "xt": xt_np,
        })
    return in_maps, assign, overflow


def _host_fragment(en_output, lengths, s, e, b, att_w, att_b, tag_w, tag_b,
                   seq_len):
    """Numpy fallback for (vanishingly rare) slot-overflow fragments."""
    mem = en_output[b].astype(np.float64)
    ws = mem[s:e].sum(0)
    span = np.concatenate([mem[s], ws, mem[e - 1]])
    pos = np.arange(S)
    in_span = (pos >= s) & (pos < e)
    att_mask = in_span | (pos >= lengths[b])
    dis = np.where(pos < s, s - pos,
                   np.where(pos >= e, pos - e + 1, seq_len)).astype(np.float64)
    pwv = 1.0 - dis / seq_len
    fin = pwv[:, None] * mem
    v = span @ att_w.astype(np.float64)
    c = span @ att_b.astype(np.float64)
    sc = np.tanh(fin @ v + c)
    sc = np.where(att_mask, -1e4, sc)
    sc = sc - sc.max()
    a = np.exp(sc)
    a = a / a.sum()
    mix = a @ fin
    ms = np.concatenate([span, mix])
    lg = ms @ tag_w.astype(np.float64).T + tag_b.astype(np.float64)
    lg = lg - lg.max()
    return (lg - np.log(np.exp(lg).sum())).astype(np.float32)


def kernel(en_output, lengths, frag_b, frag_s, frag_e, att_w, att_b, tag_w,
           tag_b):
    global LAST_RESULT
    en_output = np.asarray(en_output, dtype=np.float32)
    lengths = np.asarray(lengths).astype(np.int64)
    frag_b = np.asarray(frag_b).astype(np.int64)
    frag_s = np.asarray(frag_s).astype(np.int64)
    frag_e = np.asarray(frag_e).astype(np.int64)
    att_w = np.asarray(att_w, dtype=np.float32)
    att_b = np.asarray(att_b, dtype=np.float32)
    tag_w = np.asarray(tag_w, dtype=np.float32)
    tag_b = np.asarray(tag_b, dtype=np.float32)

    seq_len = float(lengths[0])
    if seq_len not in _compiled:
        _compiled[seq_len] = _build(seq_len)
    nc = _compiled[seq_len]

    in_maps, assign, overflow = _host_prep(
        en_output, lengths, frag_b, frag_s, frag_e, att_w, att_b, tag_w, tag_b)

    res = run_bass_kernel_spmd(nc, in_maps, core_ids=list(range(NCORES)),
                               trace=TRACE)
    LAST_RESULT = res

    out = np.empty((F, T), dtype=np.float32)
    per_core = [res.results[i]["out"][0:Q].transpose(1, 0, 2).reshape(C, T)
                for i in range(NCORES)]
    cores = assign[:, 0]
    slots = assign[:, 1]
    for core in range(NCORES):
        sel = cores == core
        out[sel] = per_core[core][slots[sel]]
    for i in overflow:
        out[i] = _host_fragment(en_output, lengths, int(frag_s[i]),
                                int(frag_e[i]), int(frag_b[i]), att_w, att_b,
                                tag_w, tag_b, seq_len)
    return out


# revision 12
# speedup vs baseline: 1.2740x; 1.2236x over previous
"""Trainium2 Bass kernel for nn_AspectModel (span-attention aspect tagger).

Strategy: batch-shard the 32 sentences 4-per-core across 8 NeuronCores; route
each fragment (host-side) to the core owning its sentence, padded to 48 slots
per sentence (192 slots/core).  All heavy math runs on-chip:
  - span features (l_word / word_state / r_word) via a masks-matmul against
    the sentence hidden states (host-built one-hot + in-span masks),
  - v = span @ att_w and c = span @ att_b as dense matmuls over all slots,
  - attention scores via a PE matmul of V against the transposed memory,
  - masked softmax (fused exp+sum) and mix via a second masks-matmul,
  - tag logits + log_softmax (the final ln computed on the DVE via an
    exponent-extraction polynomial, keeping the scalar engine on a single
    activation table -- tanh/exp/copy -- with zero mid-kernel reloads).
Matmul operands are pre-cast to bf16 on the host; f32 PSUM accumulation
keeps precision.  DMA layout is tuned for the SDMA engines (per-partition
packets pipeline only when several DMAs are outstanding, so the input
stream is split into 7 fat-row DMAs) and for the tile framework's 8 DMA
semaphore lanes.  The memory transpose is host-precomputed (the on-chip
xbar transpose emits 256B packets and runs 3x slower than streaming from
HBM).  Position weights are computed directly in [s, slot] layout: fragment
starts/ends are recovered on-chip from the one-hot masks via iota matmuls
whose broadcast-stationary form also replicates them across partitions, so
no meta tensor, no pw transpose and no gpsimd broadcast is needed.
Each core returns its own [2, 96, 5] slot outputs; the host scatters them
back into the full [1024, 5] output.  No collectives needed.
"""

import sys
import types

import ml_dtypes
import numpy as np

# Optional shim so run_bass_kernel_spmd(trace=True) works in containers where
# antenv.axon_hooks is missing (profiling only; correctness path unaffected).
try:
    import antenv.axon_hooks  # noqa: F401
except ImportError:
    try:
        from trn_agent_boot.trn_boot import _ntff_profile_via_ctypes

        _hook = _ntff_profile_via_ctypes("/opt/axon/libaxon_pjrt.so")
        _mod = types.ModuleType("antenv.axon_hooks")
        _mod.get_axon_ntff_profile_hook = lambda: _hook
        _mod.set_axon_ntff_profile_hook = lambda h: None
        sys.modules["antenv.axon_hooks"] = _mod
    except Exception:
        pass

import concourse.bass as bass  # noqa: E402
import concourse.tile as tile  # noqa: E402
from concourse import bacc, mybir  # noqa: E402
from concourse import bass_utils  # noqa: E402
from concourse.bass_utils import run_bass_kernel_spmd  # noqa: E402

# No artifact bucket in the sandbox; make tracing's upload step a no-op.
bass_utils.upload_artifacts = lambda tmpdir: f"local:{tmpdir}"

F32 = mybir.dt.float32
BF16 = mybir.dt.bfloat16
I32 = mybir.dt.int32
ALU = mybir.AluOpType
ACT = mybir.ActivationFunctionType

B, S, D, F, T = 32, 256, 512, 1024, 5
NCORES = 8
SEN = 4          # sentences per core
G = 48           # fragment slots per sentence
C = SEN * G      # 192 fragment slots per core
Q = C // 2       # 96 slots per q-half (sentence pair)
D3 = 3 * D
MKW = 2 * 4 * C      # mask columns in the sx tensor (1536)
XW = 2 * SEN * D     # x columns in the sx tensor (4096)
AWTW = 12 + 16 * T + T  # awt cols: ab | tag_w.T | tag_b row

# fast-ln constants: ln(x) = K1*(bits(x)-B) + g*(A + Bc*f),  g = f - f^2
LN_B = 1065353216
LN_K1 = float(np.log(2.0) / (1 << 23))
LN_A = float(0.4228442669815884 * np.log(2.0))
LN_BC = float(-0.15917172953269842 * np.log(2.0))
LN_SCALE = float(2.0 ** -23)

TRACE = False
LAST_RESULT = None  # BassKernelResults of the most recent run (for test.py)

_compiled = {}


def _build(seq_len: float):
    """Build + compile the per-core SPMD graph (identical on all 8 cores)."""
    nc = bacc.Bacc("TRN2", target_bir_lowering=False, debug=False,
                   num_devices=NCORES)

    # sx = [span masks (k, comp, slot) | x (chunk, d)] -- fat-row DMAs.
    sx_d = nc.dram_tensor("sx", [128, MKW + XW], BF16, kind="ExternalInput")
    aw_d = nc.dram_tensor("aw", [128, 12, D], BF16, kind="ExternalInput")
    # awt = [ab (12) | tag_w.T packed (16*5) | row0: tag_b (5)]
    awt_d = nc.dram_tensor("awt", [128, AWTW], BF16, kind="ExternalInput")
    # host-transposed memory [d%128, (l, k), (dj, s')]
    xt_d = nc.dram_tensor("xt", [128, 2 * SEN, 512], BF16,
                          kind="ExternalInput")
    out_d = nc.dram_tensor("out", [128, 2, T], F32, kind="ExternalOutput")

    with tile.TileContext(nc) as tc:
        with (
            tc.tile_pool(name="persist", bufs=1) as pp,
            tc.tile_pool(name="work", bufs=2) as wp,
            tc.tile_pool(name="psum", bufs=2, space="PSUM") as psp,
        ):
            # ---- persistent SBUF tensors ----
            sx_sb = pp.tile([128, MKW + XW], BF16, tag="sx_sb")
            mkT_sb = sx_sb[:, 0:MKW].rearrange(
                "p (k c s) -> p k c s", k=2, c=4)
            mkT = [mkT_sb[:, k, 0:3, :] for k in range(2)]
            x_bf = sx_sb[:, MKW:MKW + XW].rearrange(
                "p (m d) -> p m d", m=2 * SEN)
            aw_bf = pp.tile([128, 12, D], BF16, tag="aw_bf")
            awt_bf = pp.tile([128, AWTW], BF16, tag="awt_bf")
            tb_bf = awt_bf[0:1, 92:92 + T]
            spanT = pp.tile([128, 12, C], BF16, tag="spanT")
            v_sb = pp.tile([128, 4, C], BF16, tag="v_sb")
            memT = pp.tile([128, SEN, 2, 4, 128], BF16, tag="memT")
            mixT = pp.tile([128, 4, C], BF16, tag="mixT")

            # ---- input DMAs.  SDMA engines only pipeline packets when
            # several DMAs are outstanding, so the stream is split into 7
            # fat-row DMAs (sync ring, consumer-priority order; awt on the
            # scalar ring).  7 inputs + 2 outputs fits the 8 DMA semaphore
            # lanes without mid-stream lane-retirement stalls.
            d1 = nc.sync.dma_start(sx_sb[:, 0:MKW + XW // 2],
                                   sx_d.ap()[:, 0:MKW + XW // 2])
            prev = d1
            for dd in [
                nc.sync.dma_start(sx_sb[:, MKW + XW // 2:],
                                  sx_d.ap()[:, MKW + XW // 2:]),
                nc.sync.dma_start(aw_bf[:, 0:6, :], aw_d.ap()[:, 0:6, :]),
                nc.sync.dma_start(aw_bf[:, 6:12, :], aw_d.ap()[:, 6:12, :]),
                nc.sync.dma_start(memT[:, 0:2, :, :, :],
                                  xt_d.ap()[:, 0:4, :]),
                nc.sync.dma_start(memT[:, 2:4, :, :, :],
                                  xt_d.ap()[:, 4:8, :]),
            ]:
                tile.add_dep_helper(dd.ins, prev.ins, sync=False,
                                    reason="sync ring order")
                prev = dd
            nc.scalar.dma_start(awt_bf[:], awt_d.ap())

            # ---- constants ----
            neg4 = pp.tile([128, 1], F32, tag="neg4")
            nc.gpsimd.memset(neg4[:], -1.0e4)
            one1 = pp.tile([1, 1], F32, tag="one1")
            nc.gpsimd.memset(one1[:], 1.0)
            ones_bf = pp.tile([128, 1], BF16, tag="ones_bf")
            nc.gpsimd.memset(ones_bf[:], 1.0)
            ones_row = pp.tile([1, 128], BF16, tag="ones_row")
            nc.gpsimd.memset(ones_row[:], 1.0)
            # Warm the scalar-engine activation table while DMAs stream;
            # the scalar engine only ever runs tanh/exp/copy (one table).
            dmy = wp.tile([1, 1], F32, tag="dmy")
            nc.scalar.activation(dmy[:], one1[:], ACT.Tanh)
            nc.scalar.activation(dmy[:], one1[:], ACT.Exp)
            # iota over partitions: s_col[k] = p + 128k (f32 + exact bf16)
            iota_i = pp.tile([128, 1], I32, tag="iota_i")
            nc.gpsimd.iota(iota_i[:], pattern=[[0, 1]], channel_multiplier=1)
            s_col = pp.tile([128, 2], F32, tag="s_col")
            nc.vector.tensor_copy(s_col[:, 0:1], iota_i[:])
            nc.vector.tensor_scalar(s_col[:, 1:2], s_col[:, 0:1], 128.0, None,
                                    op0=ALU.add)
            s_bf = pp.tile([128, 2], BF16, tag="s_bf")
            nc.vector.tensor_copy(s_bf[:], s_col[:])

            # ---- fs/fm rows from the one-hot masks via iota matmuls; the
            # broadcast-stationary form replicates the row across all 128
            # output partitions (psum-resident, read directly by the DVE).
            fs_bc = psp.tile([128, C], F32, tag="pout", bufs=4)
            for k in range(2):
                nc.tensor.matmul(
                    fs_bc[:], s_bf[:, k:k + 1].broadcast_to([128, 128]),
                    mkT_sb[:, k, 0, :], start=(k == 0), stop=(k == 1))
            fm_bc = psp.tile([128, C], F32, tag="pout", bufs=4)
            for k in range(2):
                nc.tensor.matmul(
                    fm_bc[:], s_bf[:, k:k + 1].broadcast_to([128, 128]),
                    mkT_sb[:, k, 2, :], start=(k == 0), stop=(k == 1))

            # ---- span masks-matmul: spanT[3D, C] (l_word | word_state | r_word)
            # j0-major so the first half of spanT's kk chunks completes early
            # and vmm can begin while the second half is still accumulating.
            # PSUM->SBUF casts live on the scalar engine (idle here; keeps
            # the DVE free for the pw pipeline).
            sc_span = nc.named_scope("spanmm"); sc_span.__enter__()
            for j0 in range(2):
                for l in range(SEN):
                    ps = psp.tile([128, 2, 3, G], F32, tag="psm")
                    for dj in range(2):
                        j = j0 * 2 + dj
                        for k in range(2):
                            nc.tensor.matmul(
                                ps[:, dj, :, :],
                                x_bf[:, l * 2 + k, j * 128:(j + 1) * 128],
                                mkT[k][:, :, l * G:(l + 1) * G],
                                start=(k == 0), stop=(k == 1),
                            )
                    nc.scalar.copy(
                        spanT[:, j0 * 6:j0 * 6 + 6, l * G:(l + 1) * G], ps[:])

            sc_span.__exit__(None, None, None)
            # ---- pos-weight directly in [s', k, slot] layout (no transpose)
            pw_t = pp.tile([128, 2, C], BF16, tag="pw_t")
            for k in range(2):
                sk = s_col[:, k:k + 1]
                t1 = wp.tile([128, C], F32, tag="t1")
                t2 = wp.tile([128, C], F32, tag="t2")
                dm = wp.tile([128, C], F32, tag="dm")
                pwr = wp.tile([128, C], F32, tag="pwr")
                noti = wp.tile([128, C], F32, tag="noti")
                nc.vector.tensor_scalar(t1[:], fs_bc[:], sk, None,
                                        op0=ALU.subtract)        # fs - s
                nc.vector.tensor_scalar(t2[:], fm_bc[:], sk, -1.0,
                                        op0=ALU.subtract,
                                        op1=ALU.mult)            # s - fm
                nc.vector.tensor_tensor(dm[:], t1[:], t2[:], op=ALU.max)
                nc.vector.tensor_scalar(pwr[:], dm[:], -1.0 / seq_len, 1.0,
                                        op0=ALU.mult, op1=ALU.add)
                nc.vector.tensor_single_scalar(noti[:], dm[:], 0.0,
                                               op=ALU.is_gt)     # not in span
                nc.vector.tensor_tensor(pw_t[:, k, :], pwr[:], noti[:],
                                        op=ALU.mult)

            # ---- c = span @ att_b as a row, then PE-broadcast to [128, C]
            # (before vmm: cmm only needs awt + spanT, fills the PE while the
            # att_w stream is still in flight)
            sc_c = nc.named_scope("cmm"); sc_c.__enter__()
            pc = psp.tile([1, C], F32, tag="pout", bufs=4)
            for kk in range(12):
                nc.tensor.matmul(
                    pc[:],
                    awt_bf[:, kk:kk + 1],
                    spanT[:, kk, :],
                    start=(kk == 0), stop=(kk == 11),
                )
            c_row = pp.tile([1, C], F32, tag="c_row")
            nc.vector.tensor_copy(c_row[:], pc[:])
            c_bf = pp.tile([1, C], BF16, tag="c_bf")
            nc.vector.tensor_copy(c_bf[:], c_row[:])
            c_bc = psp.tile([128, C], F32, tag="pout", bufs=4)
            nc.tensor.matmul(c_bc[:], ones_row[:], c_bf[:],
                             start=True, stop=True)

            sc_c.__exit__(None, None, None)
            # ---- v = span @ att_w  (stored transposed: V[d, slot])
            sc_v = nc.named_scope("vmm"); sc_v.__enter__()
            for m0 in range(2):
                pv = psp.tile([128, 2, C], F32, tag="p2k")
                for mj in range(2):
                    m = m0 * 2 + mj
                    for kk in range(12):
                        nc.tensor.matmul(
                            pv[:, mj, :],
                            aw_bf[:, kk, m * 128:(m + 1) * 128],
                            spanT[:, kk, :],
                            start=(kk == 0), stop=(kk == 11),
                        )
                nc.vector.tensor_copy(v_sb[:, m0 * 2:m0 * 2 + 2, :], pv[:])

            sc_v.__exit__(None, None, None)
            # ---- per sentence-pair attention + mix + logits ----
            # scores kept transposed ([s, slot]); softmax denominator via a
            # PE ones-matmul; normalization folded into the logits scale.
            sh_t = pp.tile([128, 2, T], F32, tag="sh_t")
            se_t = pp.tile([128, 2], F32, tag="se_t")
            res_t = pp.tile([128, 2, T], F32, tag="res_t")

            # PE-early: both pairs' score matmuls and span-logits (with
            # tag_b folded in via a ones-row matmul), so the PE FIFO never
            # stalls behind the pair-0 activation chain.
            gts = []
            for q in range(2):
                gt = psp.tile([128, 2, Q], F32, tag="p2k", name=f"gt{q}")
                gts.append(gt)
                for h in range(2):
                    l = 2 * q + h
                    for k in range(2):
                        for dj in range(4):
                            nc.tensor.matmul(
                                gt[:, k, h * G:(h + 1) * G],
                                memT[:, l, k, dj, :],
                                v_sb[:, dj, l * G:(l + 1) * G],
                                start=(dj == 0), stop=(dj == 3),
                            )
            plss = []
            for q in range(2):
                pls = psp.tile([128, T], F32, tag="pout", bufs=4, name=f"pls{q}")
                for kk in range(12):
                    nc.tensor.matmul(
                        pls[0:Q, :], spanT[:, kk, q * Q:(q + 1) * Q],
                        awt_bf[:, 12 + kk * T:12 + (kk + 1) * T],
                        start=(kk == 0), stop=False)
                nc.tensor.matmul(pls[0:Q, :], ones_row[0:1, 0:Q], tb_bf,
                                 start=False, stop=True)
                pls_sb = pp.tile([128, T], F32, tag=f"pls_sb{q}")
                nc.vector.tensor_copy(pls_sb[0:Q, :], pls[0:Q, :])
                plss.append(pls_sb)

            for q in range(2):
                gt = gts[q]
                sg = wp.tile([128, 2, Q], F32, tag="sg")
                th = wp.tile([128, 2, Q], F32, tag="th")
                thm = wp.tile([128, 2, Q], F32, tag="thm")
                uT = wp.tile([128, 2, Q], BF16, tag="uT")
                wTu = wp.tile([128, 2, Q], BF16, tag="wTu")
                rden = wp.tile([128, 1], F32, tag="rden")
                pwq = pw_t[:, :, q * Q:(q + 1) * Q]

                # scores = tanh(pw * G + c); masked exp (still un-normalized)
                nc.vector.tensor_tensor(sg[:], gt[:], pwq, op=ALU.mult)
                cb = c_bc[:, q * Q:(q + 1) * Q]
                nc.vector.tensor_tensor(
                    sg[:], sg[:],
                    cb.rearrange("p (o c) -> p o c",
                                 o=1).broadcast_to([128, 2, Q]),
                    op=ALU.add)
                nc.scalar.activation(th[:], sg[:], ACT.Tanh)
                kpq = mkT_sb[:, :, 3, q * Q:(q + 1) * Q]
                nc.vector.scalar_tensor_tensor(thm[:], th[:], 1.0e4, kpq,
                                               op0=ALU.add, op1=ALU.mult)
                nc.scalar.activation(uT[:], thm[:], ACT.Exp, bias=neg4[:])

                # denominator via ones-matmul ([slot, 1] per pair)
                dn = psp.tile([128, 1], F32, tag="pout", bufs=4)
                for k in range(2):
                    nc.tensor.matmul(dn[0:Q, :], uT[:, k, :], ones_bf[:],
                                     start=(k == 0), stop=(k == 1))
                nc.vector.reciprocal(rden[0:Q, :], dn[0:Q, :])
                nc.vector.tensor_tensor(wTu[:], uT[:], pwq, op=ALU.mult)

                # mixT_unnorm[d, slot] = sum_s mem[s, d] * u[slot, s] * pw
                for h in range(2):
                    l = 2 * q + h
                    pm = psp.tile([128, 4, G], F32, tag="psm")
                    for dj in range(4):
                        for k in range(2):
                            nc.tensor.matmul(
                                pm[:, dj, :],
                                x_bf[:, l * 2 + k, dj * 128:(dj + 1) * 128],
                                wTu[:, k, h * G:(h + 1) * G],
                                start=(k == 0), stop=(k == 1),
                            )
                    nc.vector.tensor_copy(mixT[:, :, l * G:(l + 1) * G], pm[:])

                plm = psp.tile([128, T], F32, tag="pout", bufs=4)
                for dj in range(4):
                    nc.tensor.matmul(
                        plm[0:Q, :], mixT[:, dj, q * Q:(q + 1) * Q],
                        awt_bf[:, 12 + (12 + dj) * T:12 + (13 + dj) * T],
                        start=(dj == 0), stop=(dj == 3))

                # logits = pls(+tb) + rden*plm; log-softmax without the max
                # shift (logits are small; exp is safe in f32).  Whole
                # epilogue is per-q so pair 0's result DMA overlaps pair 1.
                nc.vector.scalar_tensor_tensor(
                    sh_t[0:Q, q, :], plm[0:Q, :], rden[0:Q, :],
                    plss[q][0:Q, :], op0=ALU.mult, op1=ALU.add)
                ex2 = wp.tile([128, T], F32, tag="ex2")
                nc.scalar.activation(ex2[0:Q, :], sh_t[0:Q, q, :], ACT.Exp,
                                     accum_out=se_t[0:Q, q:q + 1])
                # ln(se) on the DVE: exponent extraction + cubic mantissa fit
                se_bits = se_t[0:Q, q:q + 1].bitcast(I32)
                ti_i = wp.tile([128, 1], I32, tag="ti_i")
                tif = wp.tile([128, 1], F32, tag="tif")
                fi_i = wp.tile([128, 1], I32, tag="fi_i")
                ff = wp.tile([128, 1], F32, tag="ff")
                fv = wp.tile([128, 1], F32, tag="fv")
                nf2 = wp.tile([128, 1], F32, tag="nf2")
                gg = wp.tile([128, 1], F32, tag="gg")
                ab = wp.tile([128, 1], F32, tag="ab")
                corr = wp.tile([128, 1], F32, tag="corr")
                lse1 = wp.tile([128, 1], F32, tag="lse1")
                nc.vector.tensor_scalar(ti_i[0:Q, :], se_bits, LN_B, None,
                                        op0=ALU.subtract)
                nc.vector.tensor_copy(tif[0:Q, :], ti_i[0:Q, :])
                nc.vector.tensor_scalar(fi_i[0:Q, :], se_bits, 0x7FFFFF,
                                        None, op0=ALU.bitwise_and)
                nc.vector.tensor_copy(ff[0:Q, :], fi_i[0:Q, :])
                nc.vector.tensor_scalar(fv[0:Q, :], ff[0:Q, :], LN_SCALE,
                                        None, op0=ALU.mult)
                nc.vector.scalar_tensor_tensor(nf2[0:Q, :], fv[0:Q, :], -1.0,
                                               fv[0:Q, :], op0=ALU.mult,
                                               op1=ALU.mult)
                nc.vector.tensor_tensor(gg[0:Q, :], fv[0:Q, :], nf2[0:Q, :],
                                        op=ALU.add)
                nc.vector.tensor_scalar(ab[0:Q, :], fv[0:Q, :], LN_BC, LN_A,
                                        op0=ALU.mult, op1=ALU.add)
                nc.vector.tensor_tensor(corr[0:Q, :], gg[0:Q, :], ab[0:Q, :],
                                        op=ALU.mult)
                nc.vector.scalar_tensor_tensor(lse1[0:Q, :], tif[0:Q, :],
                                               LN_K1, corr[0:Q, :],
                                               op0=ALU.mult, op1=ALU.add)
                nc.vector.tensor_scalar(res_t[0:Q, q, :], sh_t[0:Q, q, :],
                                        lse1[0:Q, :], None, op0=ALU.subtract)
                nc.sync.dma_start(out_d.ap()[0:Q, q, :], res_t[0:Q, q, :])

    nc.compile()
    return nc


def _host_prep(en_output, lengths, frag_b, frag_s, frag_e, att_w, att_b,
               tag_w, tag_b):
    """Shard + relayout inputs.  Returns (in_maps, assign, overflow)."""
    # replicated weights, permuted so spanT chunk kk = 3*j + comp maps to
    # att rows comp*512 + j*128 : .. + 128.
    perm = np.concatenate([
        np.arange(comp * D + j * 128, comp * D + (j + 1) * 128)
        for j in range(4) for comp in range(3)
    ])
    aw_np = att_w[perm].reshape(12, 128, D).transpose(1, 0, 2).reshape(128, 12, D)
    ab_np = att_b[perm].reshape(12, 128).T.copy()
    tw_rows = np.concatenate([tag_w[:, perm].T,
                              tag_w[:, D3:].T], axis=0)  # [2048, 5]
    tw_np = tw_rows.reshape(16, 128, T).transpose(1, 0, 2).reshape(128, 16, T)

    aw_np = np.ascontiguousarray(aw_np).astype(ml_dtypes.bfloat16)
    awt_np = np.zeros((128, AWTW), np.float32)
    awt_np[:, 0:12] = ab_np
    awt_np[:, 12:92] = tw_np.reshape(128, 16 * T)
    awt_np[0, 92:92 + T] = tag_b
    awt_np = np.ascontiguousarray(awt_np).astype(ml_dtypes.bfloat16)

    assign = np.full((F, 2), -1, dtype=np.int64)  # (core, slot) per fragment
    counts = np.zeros((NCORES, SEN), dtype=np.int64)
    overflow = []
    in_maps = []

    fs_slot = np.zeros((NCORES, C), np.float32)
    fm_slot = np.zeros((NCORES, C), np.float32)
    ln_slot = np.full((NCORES, C), float(S), np.float32)

    for i in range(F):
        b = int(frag_b[i])
        core, l = b // SEN, b % SEN
        k = counts[core, l]
        if k >= G:
            overflow.append(i)
            continue
        counts[core, l] += 1
        slot = l * G + k
        assign[i] = (core, slot)
        fs_slot[core, slot] = frag_s[i]
        fm_slot[core, slot] = frag_e[i] - 1
        ln_slot[core, slot] = lengths[b]

    for core in range(NCORES):
        xs = np.asarray(en_output[core * SEN:(core + 1) * SEN])  # [4, 256, 512]
        x_np = xs.reshape(SEN, 2, 128, D).transpose(2, 0, 1, 3) \
                 .reshape(128, 2 * SEN * D)
        # [d%128, (l, k), (dj, s')]
        xt_np = np.ascontiguousarray(
            xs.reshape(SEN, 2, 128, 4, 128).transpose(4, 0, 1, 3, 2)
              .reshape(128, 2 * SEN, 512)).astype(ml_dtypes.bfloat16)
        # span masks [S, 4, C] -> [128, (k=2, comp=4, C)] (s = k*128 + p);
        # component 3 is the attention keep-mask (!in_span & s < len)
        pos = np.arange(S, dtype=np.float32)[:, None]
        fs = fs_slot[core][None, :]
        fm = fm_slot[core][None, :]
        ln = ln_slot[core][None, :]
        mk = np.empty((S, 4, C), np.float32)
        in_span = (pos >= fs) & (pos <= fm)
        mk[:, 0, :] = pos == fs
        mk[:, 1, :] = in_span
        mk[:, 2, :] = pos == fm
        mk[:, 3, :] = (~in_span) & (pos < ln)
        mk = mk.reshape(2, 128, 4 * C).transpose(1, 0, 2).reshape(128, 2 * 4 * C)
        sx_np = np.ascontiguousarray(np.concatenate(
            [mk, x_np], axis=1)).astype(ml_dtypes.bfloat16)
        in_maps.append({
            "sx": sx_np, "aw": aw_np, "awt": awt_np, "xt": xt_np,
        })
    return in_maps, assign, overflow


def _host_fragment(en_output, lengths, s, e, b, att_w, att_b, tag_w, tag_b,
                   seq_len):
    """Numpy fallback for (vanishingly rare) slot-overflow fragments."""
    mem = en_output[b].astype(np.float64)
    ws = mem[s:e].sum(0)
    span = np.concatenate([mem[s], ws, mem[e - 1]])
    pos = np.arange(S)
    in_span = (pos >= s) & (pos < e)
    att_mask = in_span | (pos >= lengths[b])
    dis = np.where(pos < s, s - pos,
                   np.where(pos >= e, pos - e + 1, seq_len)).astype(np.float64)
    pwv = 1.0 - dis / seq_len
    fin = pwv[:, None] * mem
    v = span @ att_w.astype(np.float64)
    c = span @ att_b.astype(np.float64)
    sc = np.tanh(fin @ v + c)
    sc = np.where(att_mask, -1e4, sc)
    sc = sc - sc.max()
    a = np.exp(sc)
    a = a / a.sum()
    mix = a @ fin
    ms = np.concatenate([span, mix])
    lg = ms @ tag_w.astype(np.float64).T + tag_b.astype(np.float64)
    lg = lg - lg.max()
    return (lg - np.log(np.exp(lg).sum())).astype(np.float32)


def kernel(en_output, lengths, frag_b, frag_s, frag_e, att_w, att_b, tag_w,
           tag_b):
    global LAST_RESULT
    en_output = np.asarray(en_output, dtype=np.float32)
    lengths = np.asarray(lengths).astype(np.int64)
    frag_b = np.asarray(frag_b).astype(np.int64)
    frag_s = np.asarray(frag_s).astype(np.int64)
    frag_e = np.asarray(frag_e).astype(np.int64)
    att_w = np.asarray(att_w, dtype=np.float32)
    att_b = np.asarray(att_b, dtype=np.float32)
    tag_w = np.asarray(tag_w, dtype=np.float32)
    tag_b = np.asarray(tag_b, dtype=np.float32)

    seq_len = float(lengths[0])
    if seq_len not in _compiled:
        _compiled[seq_len] = _build(seq_len)
    nc = _compiled[seq_len]

    in_maps, assign, overflow = _host_prep(
        en_output, lengths, frag_b, frag_s, frag_e, att_w, att_b, tag_w, tag_b)

    res = run_bass_kernel_spmd(nc, in_maps, core_ids=list(range(NCORES)),
                               trace=TRACE)
    LAST_RESULT = res

    out = np.empty((F, T), dtype=np.float32)
    per_core = [res.results[i]["out"][0:Q].transpose(1, 0, 2).reshape(C, T)
                for i in range(NCORES)]
    cores = assign[:, 0]
    slots = assign[:, 1]
    for core in range(NCORES):
        sel = cores == core
        out[sel] = per_core[core][slots[sel]]
    for i in overflow:
        out[i] = _host_fragment(en_output, lengths, int(frag_s[i]),
                                int(frag_e[i]), int(frag_b[i]), att_w, att_b,
                                tag_w, tag_b, seq_len)
    return out
